# revision 15
# baseline (speedup 1.0000x reference)
"""nn_Encoder_48095043780825: 2-hop weighted-mean SAGEConv GNN encoder
on 8 Trainium2 NeuronCores (Bass/Tile), self-contained.

kernel(**inputs) -> np.ndarray [200000, 128] float32

Distribution (per-input JIT-specialized program; integer structure baked,
all float math on device):
 - Host relabels the 50k nodes, balanced by in-degree, into 8 cores x 49
   blocks of 128 slots (dst-sharding).  Edge weights are pre-normalized on
   host (w/segment_sum(w)[dst]) so the device skips the denominator pass.
   Edges are partitioned by destination block, packed into 128-edge chunks.
 - Per 128-edge chunk one indirect DMA gathers the source rows (bf16,
   walrus caps indirect DMAs at one offset per dest partition); gathers
   round-robin over the SWDGE queues.
 - Per chunk, the DVE builds mask[e,dst] = wn_e * (dst_local[e]==dst) in
   bf16; the PE accumulates aggT += msg.T @ mask in PSUM (bf16 matmul, 4x
   faster than fp32, and directly in [D, node] layout -- no transpose).
 - Per block: hx = Ws.T@xT + Wn.T@aggT (PE), bias+ReLU on ACT writing the
   next xT column block (bf16), PE-transpose to row layout, grouped DMA out
   (7 blocks per HWDGE descriptor batch).
 - AllGather (bf16) of the new-x shards between hops.
 - Final phase: queries in natural order; per 7-chunk group the per-chunk
   x2[qnode] indirect gathers land in one wide tile, the effect rows
   (host-staged effq = effect_emb[qeff], pure input layout) load via one
   regular DMA, one wide DVE add to fp32, one grouped DMA out.
"""
import sys
sys.path.insert(0, "/opt/trn_rl_repo")
import heapq
import os
import time
import numpy as np

import jax
from jax.sharding import Mesh, PartitionSpec
from jax.experimental.shard_map import shard_map

import ml_dtypes

from concourse import bass, mybir
from concourse.tile import TileContext
from concourse.bass2jax import (
    _bass_exec_p,
    install_neuronx_cc_hook,
    partition_id_tensor,
)

P = 128
F32 = mybir.dt.float32
BF16 = mybir.dt.bfloat16
I32 = mybir.dt.int32
BF = ml_dtypes.bfloat16

CFG = dict(N=50000, E=600000, D=128, NEFF=1000, Q=200000, C=8, B=49, HOPS=2)

GB = 7    # blocks per grouped newx write-out DMA
GQ = 7    # query chunks per final-phase group (bounds DMA descriptor count)
NQUEUES = 2


# ------------------------------------------------------------------ wait split

def _split_wide_waits(nc, max_waits=1):
    """This walrus build rejects instructions with more than one sync-wait
    command; move excess waits onto preceding NoOps on the same engine."""
    for f in nc.m.functions:
        for bb in f.blocks:
            new_instrs = []
            for ins in bb.instructions:
                si = ins.sync_info
                if si is not None and si.on_wait and len(si.on_wait) > max_waits:
                    waits = list(si.on_wait)
                    head, tail = waits[:-max_waits], waits[-max_waits:]
                    for i in range(0, len(head), max_waits):
                        nop = mybir.InstNoOp(
                            name=nc.get_next_instruction_name(),
                            engine=ins.engine,
                            ins=[], outs=[],
                            sync_info=mybir.SyncInfo(
                                on_wait=head[i:i + max_waits], on_update=[]),
                            text_hint="waitsplit",
                            bass_nofuse=True,
                        )
                        new_instrs.append(nop)
                    si.on_wait = tail
                new_instrs.append(ins)
            bb.instructions = new_instrs


# ------------------------------------------------------------------ host prep

def _balance_nodes(dst, N, n_bins):
    npad = n_bins * P
    deg = np.bincount(dst, minlength=N).astype(np.int64)
    deg_pad = np.zeros(npad, np.int64)
    deg_pad[:N] = deg
    order = np.argsort(-deg_pad, kind="stable")
    heap = [(0, b) for b in range(n_bins)]
    heapq.heapify(heap)
    counts = np.zeros(n_bins, np.int32)
    newid = np.empty(npad, np.int64)
    for n in order:
        while True:
            load, b = heapq.heappop(heap)
            if counts[b] < P:
                break
        newid[n] = b * P + counts[b]
        counts[b] += 1
        if counts[b] < P:
            heapq.heappush(heap, (load + deg_pad[n], b))
    assert counts.min() == counts.max() == P
    return newid


def _prep(inputs, cfg):
    N, E, D, NEFF, Q = cfg["N"], cfg["E"], cfg["D"], cfg["NEFF"], cfg["Q"]
    C, B = cfg["C"], cfg["B"]
    NPC = B * P
    NPAD = C * NPC
    assert NPAD >= N and D == 128

    graph_x = np.asarray(inputs["graph_x"], np.float32)
    edge_index = np.asarray(inputs["edge_index"])
    src = edge_index[0].astype(np.int64)
    dst = edge_index[1].astype(np.int64)
    w = np.asarray(inputs["chemical_similarity"], np.float32)
    x_nodes = np.asarray(inputs["x_nodes"]).astype(np.int64)
    effect_ids = np.asarray(inputs["effect_ids"]).astype(np.int64)
    W_self = np.asarray(inputs["W_self"], np.float32)
    W_neigh = np.asarray(inputs["W_neigh"], np.float32)
    bias = np.asarray(inputs["bias"], np.float32)
    effect_emb = np.asarray(inputs["effect_emb"], np.float32)

    # host-normalized edge weights: segment_sum(wn*x)[v] == agg[v]/denom[v]
    denom = np.zeros(N, np.float32)
    np.add.at(denom, dst, w)
    wn = w / np.maximum(denom[dst], np.float32(1e-12))

    newid = _balance_nodes(dst, N, C * B)

    x_full = np.zeros((NPAD, D), np.float32)
    x_full[newid[:N]] = graph_x

    nsrc = newid[src]
    ndst = newid[dst]
    ecore = ndst // NPC
    eblk = (ndst % NPC) // P
    eslot = ndst % P

    loads = np.zeros((C, B), np.int64)
    np.add.at(loads, (ecore, eblk), 1)
    C_b = np.maximum(1, -(-loads.max(axis=0) // P))
    NCH = int(C_b.sum())
    cob = np.concatenate([[0], np.cumsum(C_b)])

    esrc = np.zeros((C, P, NCH), np.int32)
    edstf = np.zeros((C, P, NCH), np.float32)
    ewf = np.zeros((C, P, NCH), np.float32)

    eorder = np.lexsort((eslot, eblk, ecore))
    so, wo, slo = nsrc[eorder], wn[eorder], eslot[eorder]
    idx = 0
    for c in range(C):
        for b in range(B):
            n = int(loads[c, b])
            sl = slice(idx, idx + n)
            idx += n
            base = cob[b] * P
            pos = base + np.arange(n)
            esrc[c, pos % P, pos // P] = so[sl]
            edstf[c, pos % P, pos // P] = slo[sl].astype(np.float32)
            ewf[c, pos % P, pos // P] = wo[sl]

    # queries, natural order; effect rows staged host-side (pure layout of
    # the effect_emb input -- no arithmetic)
    QPC = -(-Q // C)
    QPAD = -(-QPC // P) * P
    QCH = QPAD // P
    qnode = np.zeros((C, P, QCH), np.int32)
    effq = np.zeros((C, QPAD, D), np.float32)
    for c in range(C):
        lo, hi = c * QPC, min((c + 1) * QPC, Q)
        nq = hi - lo
        nod_pad = np.zeros(QPAD, np.int64)
        nod_pad[:nq] = newid[x_nodes[lo:hi]]
        qnode[c] = nod_pad.reshape(QCH, P).T
        effq[c, :nq] = effect_emb[effect_ids[lo:hi]]

    iotaF = np.tile(np.arange(P, dtype=np.float32)[None, :], (P, 1)).astype(BF)
    ident = np.eye(P, dtype=np.float32).astype(BF)

    x_full_bf = x_full.astype(BF)

    meta = dict(cfg, NPC=NPC, NPAD=NPAD, NCH=NCH, QPC=QPC, QCH=QCH,
                C_b=C_b, chunk_of_block=cob)

    in_maps = []
    for c in range(C):
        in_maps.append({
            "x0_full": x_full_bf,
            "x0T": np.ascontiguousarray(x_full[c * NPC:(c + 1) * NPC].T).astype(BF),
            "effq": effq[c].astype(BF),
            "esrc": esrc[c], "edst": edstf[c], "ew": ewf[c],
            "qnode": qnode[c],
            "iotaF": iotaF, "ident": ident,
            "Wself": W_self.astype(BF), "Wneigh": W_neigh.astype(BF),
            "biasc": np.ascontiguousarray(bias.T).astype(np.float32),
        })
    return meta, in_maps


# --------------------------------------------------------------- device build

def _build_nc(meta, n_cores=None, variant="full", krep=1, timing=False):
    C = n_cores if n_cores is not None else meta["C"]
    B, D, NEFF, HOPS = meta["B"], meta["D"], meta["NEFF"], meta["HOPS"]
    NPC, NPAD, NCH = meta["NPC"], meta["NPAD"], meta["NCH"]
    QCH = meta["QCH"]
    C_b, cob = meta["C_b"], meta["chunk_of_block"]
    no_ag = variant.startswith("no_ag")
    no_gather = "nogather" in variant

    nc = bass.Bass(trn_type="TRN2", num_devices=C, num_swdge_queues=NQUEUES)

    x0_full = nc.dram_tensor("x0_full", [NPAD, D], BF16, kind="ExternalInput")
    x0T = nc.dram_tensor("x0T", [D, NPC], BF16, kind="ExternalInput")
    effq = nc.dram_tensor("effq", [QCH * P, D], BF16, kind="ExternalInput")
    esrc = nc.dram_tensor("esrc", [P, NCH], I32, kind="ExternalInput")
    edst = nc.dram_tensor("edst", [P, NCH], F32, kind="ExternalInput")
    ew = nc.dram_tensor("ew", [P, NCH], F32, kind="ExternalInput")
    qnode = nc.dram_tensor("qnode", [P, QCH], I32, kind="ExternalInput")
    iotaF = nc.dram_tensor("iotaF", [P, P], BF16, kind="ExternalInput")
    ident = nc.dram_tensor("ident", [P, P], BF16, kind="ExternalInput")
    Wself = nc.dram_tensor("Wself", [HOPS, D, D], BF16, kind="ExternalInput")
    Wneigh = nc.dram_tensor("Wneigh", [HOPS, D, D], BF16, kind="ExternalInput")
    biasc = nc.dram_tensor("biasc", [D, HOPS], F32, kind="ExternalInput")

    newx_loc = [nc.dram_tensor(f"newx{h}_loc", [NPC, D], BF16)
                for h in range(HOPS)]
    if no_ag:
        x_shared = [nc.dram_tensor(f"x{h+1}_full", [NPAD, D], BF16)
                    for h in range(HOPS)]
    else:
        x_shared = [nc.dram_tensor(f"x{h+1}_full", [NPAD, D], BF16,
                                   addr_space="Shared")
                    for h in range(HOPS)]
    out_dram = nc.dram_tensor("out", [QCH * P, D], F32,
                              kind=None if timing else "ExternalOutput")
    probe = (nc.dram_tensor("probe", [P, 1], F32, kind="ExternalOutput")
             if timing else None)

    rg = [list(range(C))]
    qctr = [0]

    def _queue(bi):
        i = qctr[0] % NQUEUES
        if i:
            bi.ins.queue = f"qPoolDynamic{i}"
        qctr[0] += 1
        return bi

    bgroups = [(g0, min(g0 + GB, B)) for g0 in range(0, B, GB)]
    MC = max(int(cob[g1] - cob[g0]) for g0, g1 in bgroups)
    qgroups = [(j0, min(j0 + GQ, QCH)) for j0 in range(0, QCH, GQ)]
    MQ = max(j1 - j0 for j0, j1 in qgroups)

    with TileContext(nc) as tc:
        with tc.tile_pool(name="const", bufs=1) as cp:
            iF = cp.tile([P, P], BF16, tag="iF")
            nc.sync.dma_start(out=iF[:], in_=iotaF[:, :])
            idn = cp.tile([P, P], BF16, tag="idn")
            nc.sync.dma_start(out=idn[:], in_=ident[:, :])
            Ws, Wn = [], []
            for h in range(HOPS):
                t = cp.tile([P, D], BF16, tag=f"ws{h}")
                nc.sync.dma_start(out=t[:], in_=Wself[h, :, :])
                Ws.append(t)
                t = cp.tile([P, D], BF16, tag=f"wn{h}")
                nc.sync.dma_start(out=t[:], in_=Wneigh[h, :, :])
                Wn.append(t)
            bc = cp.tile([P, HOPS], F32, tag="bc")
            nc.sync.dma_start(out=bc[:], in_=biasc[:, :])
            edst_sb = cp.tile([P, NCH], F32, tag="edst")
            nc.sync.dma_start(out=edst_sb[:], in_=edst[:, :])
            ew_sb = cp.tile([P, NCH], F32, tag="ew")
            nc.sync.dma_start(out=ew_sb[:], in_=ew[:, :])
            esrc_sb = cp.tile([P, NCH], I32, tag="esrc")
            nc.sync.dma_start(out=esrc_sb[:], in_=esrc[:, :])
            qnode_sb = cp.tile([P, QCH], I32, tag="qnode")
            nc.sync.dma_start(out=qnode_sb[:], in_=qnode[:, :])
            xT = [cp.tile([P, NPC], BF16, tag=f"xT{i}", name=f"xT{i}")
                  for i in range(2)]
            nc.sync.dma_start(out=xT[0][:], in_=x0T[:, :])

            if timing:
                # keep hop-1/final gather sources finite in every rep
                for h in range(HOPS):
                    nc.sync.dma_start(out=x_shared[h][:, :], in_=x0_full[:, :])

            for rep in range(krep):
                for h in range(HOPS):
                    table = x0_full if h == 0 else x_shared[h - 1]
                    xT_cur, xT_nxt = xT[h % 2], xT[(h + 1) % 2]
                    with tc.tile_pool(name=f"hop{h}_{rep}", bufs=12) as hp, \
                         tc.tile_pool(name=f"hopm{h}_{rep}", bufs=8) as hm, \
                         tc.tile_pool(name=f"hopb{h}_{rep}", bufs=3) as hb, \
                         tc.tile_pool(name=f"hopp{h}_{rep}", bufs=2,
                                      space="PSUM") as pp:
                        for (g0, g1) in bgroups:
                            nxw = hb.tile([P, GB * D], BF16, tag="nxw")
                            for b in range(g0, g1):
                                aggT_ps = pp.tile([P, P], F32, tag="agg",
                                                  space="PSUM")
                                nch = int(C_b[b])
                                for k in range(nch):
                                    i = int(cob[b]) + k
                                    msg = hp.tile([P, D], BF16, tag="msg")
                                    _queue(nc.gpsimd.indirect_dma_start(
                                        out=msg[:], out_offset=None,
                                        in_=table[:],
                                        in_offset=bass.IndirectOffsetOnAxis(
                                            ap=esrc_sb[:, i:i + 1], axis=0)))
                                    mask = hm.tile([P, P], BF16, tag="mask")
                                    nc.vector.tensor_scalar(
                                        out=mask[:], in0=iF[:],
                                        scalar1=edst_sb[:, i:i + 1],
                                        scalar2=ew_sb[:, i:i + 1],
                                        op0=mybir.AluOpType.is_equal,
                                        op1=mybir.AluOpType.mult)
                                    nc.tensor.matmul(
                                        aggT_ps[:],
                                        lhsT=msg[:],
                                        rhs=mask[:],
                                        start=(k == 0), stop=(k == nch - 1))
                                aggT = hb.tile([P, P], BF16, tag="aggT")
                                nc.vector.tensor_copy(aggT[:], aggT_ps[:])
                                hx_ps = pp.tile([P, P], F32, tag="hx",
                                                space="PSUM")
                                nc.tensor.matmul(hx_ps[:], lhsT=Ws[h][:],
                                                 rhs=xT_cur[:, b * P:(b + 1) * P],
                                                 start=True, stop=False)
                                nc.tensor.matmul(hx_ps[:], lhsT=Wn[h][:],
                                                 rhs=aggT[:],
                                                 start=False, stop=True)
                                nc.scalar.activation(
                                    xT_nxt[:, b * P:(b + 1) * P], hx_ps[:],
                                    mybir.ActivationFunctionType.Relu,
                                    bias=bc[:, h:h + 1])
                                nx_ps = pp.tile([P, P], BF16, tag="nx",
                                                space="PSUM")
                                nc.tensor.transpose(
                                    out=nx_ps[:],
                                    in_=xT_nxt[:, b * P:(b + 1) * P],
                                    identity=idn[:])
                                nc.vector.tensor_copy(
                                    nxw[:, (b - g0) * D:(b - g0 + 1) * D],
                                    nx_ps[:])
                            nc.sync.dma_start(
                                out=newx_loc[h][g0 * P:g1 * P, :].rearrange(
                                    "(g p) d -> p g d", p=P),
                                in_=nxw[:, :(g1 - g0) * D].rearrange(
                                    "p (g d) -> p g d", d=D))
                    if no_ag:
                        nc.sync.dma_start(out=x_shared[h][0:NPC, :],
                                          in_=newx_loc[h][:, :])
                    else:
                        nc.gpsimd.collective_compute(
                            "AllGather", mybir.AluOpType.bypass,
                            replica_groups=rg,
                            ins=[newx_loc[h][:]],
                            outs=[x_shared[h][:]])

                with tc.tile_pool(name=f"fin_{rep}", bufs=2) as fp:
                    for (j0, j1) in qgroups:
                        jq = j1 - j0
                        x2g = fp.tile([P, MQ * D], BF16, tag="x2g")
                        for j in range(j0, j1):
                            _queue(nc.gpsimd.indirect_dma_start(
                                out=x2g[:, (j - j0) * D:(j - j0 + 1) * D],
                                out_offset=None,
                                in_=x_shared[HOPS - 1][:],
                                in_offset=bass.IndirectOffsetOnAxis(
                                    ap=qnode_sb[:, j:j + 1], axis=0)))
                        efw = fp.tile([P, MQ * D], BF16, tag="efw")
                        nc.sync.dma_start(
                            out=efw[:, :jq * D].rearrange(
                                "p (g d) -> p g d", d=D),
                            in_=effq[j0 * P:j1 * P, :].rearrange(
                                "(g p) d -> p g d", p=P))
                        osb = fp.tile([P, MQ * D], F32, tag="osb")
                        nc.vector.tensor_add(osb[:, :jq * D], x2g[:, :jq * D],
                                             efw[:, :jq * D])
                        nc.sync.dma_start(
                            out=out_dram[j0 * P:j1 * P, :].rearrange(
                                "(g p) d -> p g d", p=P),
                            in_=osb[:, :jq * D].rearrange(
                                "p (g d) -> p g d", d=D))
                        if timing and rep == krep - 1 and j1 == QCH:
                            pt = fp.tile([P, 1], F32, tag="probe")
                            nc.vector.tensor_copy(pt[:], osb[:, :1])
                            nc.sync.dma_start(out=probe[:, :], in_=pt[:])
    return nc


# ------------------------------------------------------------------- runner

def _build_runner(nc, n_cores, sim=False):
    install_neuronx_cc_hook()
    partition_name = nc.partition_id_tensor.name if nc.partition_id_tensor else None

    in_names, out_names, out_avals = [], [], []
    for alloc in nc.m.functions[0].allocations:
        if not isinstance(alloc, mybir.MemoryLocationSet):
            continue
        name = alloc.memorylocations[0].name
        if alloc.kind == "ExternalInput":
            if name != partition_name:
                in_names.append(name)
        elif alloc.kind == "ExternalOutput":
            out_names.append(name)
            out_avals.append(jax.core.ShapedArray(
                tuple(alloc.tensor_shape), mybir.dt.np(alloc.dtype)))

    n_params = len(in_names)
    n_outs = len(out_avals)
    all_in_names = list(in_names) + list(out_names)
    if partition_name is not None:
        all_in_names.append(partition_name)

    def _body(*args):
        operands = list(args)
        if partition_name is not None:
            operands.append(partition_id_tensor())
        outs = _bass_exec_p.bind(
            *operands,
            out_avals=tuple(out_avals),
            in_names=tuple(all_in_names),
            out_names=tuple(out_names),
            lowering_input_output_aliases=(),
            sim_require_finite=True,
            sim_require_nnan=True,
            nc=nc,
        )
        return tuple(outs)

    devices = (jax.devices("cpu") if sim else jax.devices())[:n_cores]
    mesh = Mesh(np.asarray(devices), ("core",))
    in_specs = (PartitionSpec("core"),) * (n_params + n_outs)
    out_specs = (PartitionSpec("core"),) * n_outs
    sharded = jax.jit(
        shard_map(_body, mesh=mesh, in_specs=in_specs, out_specs=out_specs,
                  check_rep=False),
        keep_unused=True,
    )

    def _concat(in_maps):
        per_core = [[np.asarray(m[name]) for name in in_names] for m in in_maps]
        concat_in = [
            np.concatenate([per_core[c][i] for c in range(n_cores)], axis=0)
            for i in range(n_params)
        ]
        concat_zeros = [
            np.zeros((n_cores * av.shape[0], *av.shape[1:]), av.dtype)
            for av in out_avals
        ]
        return concat_in, concat_zeros

    def run(in_maps):
        concat_in, concat_zeros = _concat(in_maps)
        out_arrs = sharded(*concat_in, *concat_zeros)
        jax.block_until_ready(out_arrs)
        return [
            {name: np.asarray(out_arrs[i]).reshape(
                n_cores, *out_avals[i].shape)[c]
             for i, name in enumerate(out_names)}
            for c in range(n_cores)
        ]

    def timeit(in_maps, reps=1):
        concat_in, concat_zeros = _concat(in_maps)
        times = []
        for _ in range(reps):
            t0 = time.perf_counter()
            out_arrs = sharded(*concat_in, *concat_zeros)
            jax.block_until_ready(out_arrs)
            times.append(time.perf_counter() - t0)
        return out_arrs, times

    return run, timeit


# ------------------------------------------------------------------- kernel

def kernel(**inputs):
    gx = np.asarray(inputs["graph_x"])
    cfg = dict(
        N=gx.shape[0],
        E=np.asarray(inputs["edge_index"]).shape[1],
        D=gx.shape[1],
        NEFF=np.asarray(inputs["effect_emb"]).shape[0],
        Q=np.asarray(inputs["x_nodes"]).shape[0],
        C=8,
        B=-(-gx.shape[0] // (8 * P)),
        HOPS=np.asarray(inputs["W_self"]).shape[0],
    )
    meta, in_maps = _prep(inputs, cfg)
    nc = _build_nc(meta)
    _split_wide_waits(nc, 1)
    run, _ = _build_runner(nc, cfg["C"], sim=os.environ.get("KSIM") == "1")
    results = run(in_maps)

    C, QPC, D, Q = cfg["C"], meta["QPC"], cfg["D"], cfg["Q"]
    out = np.empty((Q, D), np.float32)
    for c in range(C):
        lo, hi = c * QPC, min((c + 1) * QPC, Q)
        out[lo:hi] = results[c]["out"][:hi - lo]
    return out


# revision 19
# speedup vs baseline: 4.7965x; 4.7965x over previous
"""nn_Encoder_48095043780825: 2-hop weighted-mean SAGEConv GNN encoder
on 8 Trainium2 NeuronCores (Bass/Tile), self-contained.

kernel(**inputs) -> np.ndarray [200000, 128] float32

Distribution (per-input JIT-specialized program; integer structure baked,
all float math on device):
 - Host relabels the 50k nodes, balanced by in-degree, into 8 cores x 49
   blocks of 128 slots (dst-sharding).  Edge weights are pre-normalized on
   host (w/segment_sum(w)[dst]) so the device skips the denominator pass.
   Edges are partitioned by destination block, packed into 128-edge chunks.
 - Per 128-edge chunk one indirect DMA gathers the source rows (bf16,
   walrus caps indirect DMAs at one offset per dest partition); gathers
   round-robin over the SWDGE queues.
 - Per chunk, the DVE builds mask[e,dst] = wn_e * (dst_local[e]==dst) in
   bf16; the PE accumulates aggT += msg.T @ mask in PSUM (bf16 matmul, 4x
   faster than fp32, and directly in [D, node] layout -- no transpose).
 - Per block: hx = Ws.T@xT + Wn.T@aggT (PE), bias+ReLU on ACT writing the
   next xT column block (bf16), PE-transpose to row layout, grouped DMA out
   (7 blocks per HWDGE descriptor batch).
 - AllGather (bf16) of the new-x shards between hops.
 - Final phase: queries in natural order; per 7-chunk group the per-chunk
   x2[qnode] indirect gathers land in one wide tile, the effect rows
   (host-staged effq = effect_emb[qeff], pure input layout) load via one
   regular DMA, one wide DVE add to fp32, one grouped DMA out.
"""
import sys
sys.path.insert(0, "/opt/trn_rl_repo")
import heapq
import os
import time
import numpy as np

import jax
from jax.sharding import Mesh, PartitionSpec
from jax.experimental.shard_map import shard_map

import ml_dtypes

from concourse import bass, mybir
from concourse.tile import TileContext
from concourse.bass2jax import (
    _bass_exec_p,
    install_neuronx_cc_hook,
    partition_id_tensor,
)

P = 128
F32 = mybir.dt.float32
BF16 = mybir.dt.bfloat16
I32 = mybir.dt.int32
BF = ml_dtypes.bfloat16

CFG = dict(N=50000, E=600000, D=128, NEFF=1000, Q=200000, C=8, B=49, HOPS=2)

GB = 7    # blocks per grouped newx write-out DMA
GQ = 7    # query chunks per final-phase group (bounds DMA descriptor count)
NQUEUES = 2


# ------------------------------------------------------------------ wait split

def _split_wide_waits(nc, max_waits=1):
    """This walrus build rejects instructions with more than one sync-wait
    command; move excess waits onto preceding NoOps on the same engine."""
    for f in nc.m.functions:
        for bb in f.blocks:
            new_instrs = []
            for ins in bb.instructions:
                si = ins.sync_info
                if si is not None and si.on_wait and len(si.on_wait) > max_waits:
                    waits = list(si.on_wait)
                    head, tail = waits[:-max_waits], waits[-max_waits:]
                    for i in range(0, len(head), max_waits):
                        nop = mybir.InstNoOp(
                            name=nc.get_next_instruction_name(),
                            engine=ins.engine,
                            ins=[], outs=[],
                            sync_info=mybir.SyncInfo(
                                on_wait=head[i:i + max_waits], on_update=[]),
                            text_hint="waitsplit",
                            bass_nofuse=True,
                        )
                        new_instrs.append(nop)
                    si.on_wait = tail
                new_instrs.append(ins)
            bb.instructions = new_instrs


# ------------------------------------------------------------------ host prep

def _balance_nodes(dst, N, n_bins):
    npad = n_bins * P
    deg = np.bincount(dst, minlength=N).astype(np.int64)
    deg_pad = np.zeros(npad, np.int64)
    deg_pad[:N] = deg
    order = np.argsort(-deg_pad, kind="stable")
    heap = [(0, b) for b in range(n_bins)]
    heapq.heapify(heap)
    counts = np.zeros(n_bins, np.int32)
    newid = np.empty(npad, np.int64)
    for n in order:
        while True:
            load, b = heapq.heappop(heap)
            if counts[b] < P:
                break
        newid[n] = b * P + counts[b]
        counts[b] += 1
        if counts[b] < P:
            heapq.heappush(heap, (load + deg_pad[n], b))
    assert counts.min() == counts.max() == P
    return newid


def _prep(inputs, cfg):
    N, E, D, NEFF, Q = cfg["N"], cfg["E"], cfg["D"], cfg["NEFF"], cfg["Q"]
    C, B = cfg["C"], cfg["B"]
    NPC = B * P
    NPAD = C * NPC
    assert NPAD >= N and D == 128

    graph_x = np.asarray(inputs["graph_x"], np.float32)
    edge_index = np.asarray(inputs["edge_index"])
    src = edge_index[0].astype(np.int64)
    dst = edge_index[1].astype(np.int64)
    w = np.asarray(inputs["chemical_similarity"], np.float32)
    x_nodes = np.asarray(inputs["x_nodes"]).astype(np.int64)
    effect_ids = np.asarray(inputs["effect_ids"]).astype(np.int64)
    W_self = np.asarray(inputs["W_self"], np.float32)
    W_neigh = np.asarray(inputs["W_neigh"], np.float32)
    bias = np.asarray(inputs["bias"], np.float32)
    effect_emb = np.asarray(inputs["effect_emb"], np.float32)

    # host-normalized edge weights: segment_sum(wn*x)[v] == agg[v]/denom[v]
    denom = np.zeros(N, np.float32)
    np.add.at(denom, dst, w)
    wn = w / np.maximum(denom[dst], np.float32(1e-12))

    newid = _balance_nodes(dst, N, C * B)

    x_full = np.zeros((NPAD, D), np.float32)
    x_full[newid[:N]] = graph_x

    nsrc = newid[src]
    ndst = newid[dst]
    ecore = ndst // NPC
    eblk = (ndst % NPC) // P
    eslot = ndst % P

    loads = np.zeros((C, B), np.int64)
    np.add.at(loads, (ecore, eblk), 1)
    C_b = np.maximum(1, -(-loads.max(axis=0) // P))
    NCH = int(C_b.sum())
    cob = np.concatenate([[0], np.cumsum(C_b)])

    esrc = np.zeros((C, P, NCH), np.int32)
    edstf = np.zeros((C, P, NCH), np.float32)
    ewf = np.zeros((C, P, NCH), np.float32)

    eorder = np.lexsort((eslot, eblk, ecore))
    so, wo, slo = nsrc[eorder], wn[eorder], eslot[eorder]
    idx = 0
    for c in range(C):
        for b in range(B):
            n = int(loads[c, b])
            sl = slice(idx, idx + n)
            idx += n
            base = cob[b] * P
            pos = base + np.arange(n)
            esrc[c, pos % P, pos // P] = so[sl]
            edstf[c, pos % P, pos // P] = slo[sl].astype(np.float32)
            ewf[c, pos % P, pos // P] = wo[sl]

    # queries, natural order; effect rows staged host-side (pure layout of
    # the effect_emb input -- no arithmetic)
    QPC = -(-Q // C)
    QPAD = -(-QPC // P) * P
    QCH = QPAD // P
    qnode = np.zeros((C, P, QCH), np.int32)
    effq = np.zeros((C, QPAD, D), np.float32)
    for c in range(C):
        lo, hi = c * QPC, min((c + 1) * QPC, Q)
        nq = hi - lo
        nod_pad = np.zeros(QPAD, np.int64)
        nod_pad[:nq] = newid[x_nodes[lo:hi]]
        qnode[c] = nod_pad.reshape(QCH, P).T
        effq[c, :nq] = effect_emb[effect_ids[lo:hi]]

    iotaF = np.tile(np.arange(P, dtype=np.float32)[None, :], (P, 1)).astype(BF)
    ident = np.eye(P, dtype=np.float32).astype(BF)

    x_full_bf = x_full.astype(BF)

    meta = dict(cfg, NPC=NPC, NPAD=NPAD, NCH=NCH, QPC=QPC, QCH=QCH,
                C_b=C_b, chunk_of_block=cob)

    in_maps = []
    for c in range(C):
        in_maps.append({
            "x0_full": x_full_bf,
            "x0T": np.ascontiguousarray(x_full[c * NPC:(c + 1) * NPC].T).astype(BF),
            "effq": effq[c].astype(BF),
            "esrc": esrc[c], "edst": edstf[c], "ew": ewf[c],
            "qnode": qnode[c],
            "iotaF": iotaF, "ident": ident,
            "Wself": W_self.astype(BF), "Wneigh": W_neigh.astype(BF),
            "biasc": np.ascontiguousarray(bias.T).astype(np.float32),
        })
    return meta, in_maps


# --------------------------------------------------------------- device build

def _build_nc(meta, n_cores=None, variant="full", krep=1, timing=False):
    C = n_cores if n_cores is not None else meta["C"]
    B, D, NEFF, HOPS = meta["B"], meta["D"], meta["NEFF"], meta["HOPS"]
    NPC, NPAD, NCH = meta["NPC"], meta["NPAD"], meta["NCH"]
    QCH = meta["QCH"]
    C_b, cob = meta["C_b"], meta["chunk_of_block"]
    no_ag = variant.startswith("no_ag")
    no_gather = "nogather" in variant

    nc = bass.Bass(trn_type="TRN2", num_devices=C, num_swdge_queues=NQUEUES)

    x0_full = nc.dram_tensor("x0_full", [NPAD, D], BF16, kind="ExternalInput")
    x0T = nc.dram_tensor("x0T", [D, NPC], BF16, kind="ExternalInput")
    effq = nc.dram_tensor("effq", [QCH * P, D], BF16, kind="ExternalInput")
    esrc = nc.dram_tensor("esrc", [P, NCH], I32, kind="ExternalInput")
    edst = nc.dram_tensor("edst", [P, NCH], F32, kind="ExternalInput")
    ew = nc.dram_tensor("ew", [P, NCH], F32, kind="ExternalInput")
    qnode = nc.dram_tensor("qnode", [P, QCH], I32, kind="ExternalInput")
    iotaF = nc.dram_tensor("iotaF", [P, P], BF16, kind="ExternalInput")
    ident = nc.dram_tensor("ident", [P, P], BF16, kind="ExternalInput")
    Wself = nc.dram_tensor("Wself", [HOPS, D, D], BF16, kind="ExternalInput")
    Wneigh = nc.dram_tensor("Wneigh", [HOPS, D, D], BF16, kind="ExternalInput")
    biasc = nc.dram_tensor("biasc", [D, HOPS], F32, kind="ExternalInput")

    newx_loc = [nc.dram_tensor(f"newx{h}_loc", [NPC, D], BF16)
                for h in range(HOPS)]
    if no_ag:
        x_shared = [nc.dram_tensor(f"x{h+1}_full", [NPAD, D], BF16)
                    for h in range(HOPS)]
    else:
        x_shared = [nc.dram_tensor(f"x{h+1}_full", [NPAD, D], BF16,
                                   addr_space="Shared")
                    for h in range(HOPS)]
    if timing:
        out_dram = nc.dram_tensor("out", [QCH * P, D], F32)
    else:
        out_dram = nc.dram_tensor("out", [QCH * P, D], F32,
                                  kind="ExternalOutput")
    probe = (nc.dram_tensor("probe", [P, 1], F32, kind="ExternalOutput")
             if timing else None)

    rg = [list(range(C))]
    qctr = [0]

    def _queue(bi):
        i = qctr[0] % NQUEUES
        if i:
            bi.ins.queue = f"qPoolDynamic{i}"
        qctr[0] += 1
        return bi

    bgroups = [(g0, min(g0 + GB, B)) for g0 in range(0, B, GB)]
    MC = max(int(cob[g1] - cob[g0]) for g0, g1 in bgroups)
    qgroups = [(j0, min(j0 + GQ, QCH)) for j0 in range(0, QCH, GQ)]
    MQ = max(j1 - j0 for j0, j1 in qgroups)

    with TileContext(nc) as tc:
        with tc.tile_pool(name="const", bufs=1) as cp:
            iF = cp.tile([P, P], BF16, tag="iF")
            nc.sync.dma_start(out=iF[:], in_=iotaF[:, :])
            idn = cp.tile([P, P], BF16, tag="idn")
            nc.sync.dma_start(out=idn[:], in_=ident[:, :])
            Ws, Wn = [], []
            for h in range(HOPS):
                t = cp.tile([P, D], BF16, tag=f"ws{h}")
                nc.sync.dma_start(out=t[:], in_=Wself[h, :, :])
                Ws.append(t)
                t = cp.tile([P, D], BF16, tag=f"wn{h}")
                nc.sync.dma_start(out=t[:], in_=Wneigh[h, :, :])
                Wn.append(t)
            bc = cp.tile([P, HOPS], F32, tag="bc")
            nc.sync.dma_start(out=bc[:], in_=biasc[:, :])
            edst_sb = cp.tile([P, NCH], F32, tag="edst")
            nc.sync.dma_start(out=edst_sb[:], in_=edst[:, :])
            ew_sb = cp.tile([P, NCH], F32, tag="ew")
            nc.sync.dma_start(out=ew_sb[:], in_=ew[:, :])
            esrc_sb = cp.tile([P, NCH], I32, tag="esrc")
            nc.sync.dma_start(out=esrc_sb[:], in_=esrc[:, :])
            qnode_sb = cp.tile([P, QCH], I32, tag="qnode")
            nc.sync.dma_start(out=qnode_sb[:], in_=qnode[:, :])
            xT = [cp.tile([P, NPC], BF16, tag=f"xT{i}", name=f"xT{i}")
                  for i in range(2)]
            nc.sync.dma_start(out=xT[0][:], in_=x0T[:, :])
            cmsg = None
            if no_gather:
                cmsg = cp.tile([P, MQ * D], BF16, tag="cmsg")
                nc.vector.memset(cmsg[:], 0.25)

            if timing:
                # keep hop-1/final gather sources finite in every rep
                for h in range(HOPS):
                    nc.sync.dma_start(out=x_shared[h][:, :], in_=x0_full[:, :])

            for rep in range(krep):
                for h in range(HOPS):
                    table = x0_full if h == 0 else x_shared[h - 1]
                    xT_cur, xT_nxt = xT[h % 2], xT[(h + 1) % 2]
                    with tc.tile_pool(name=f"hop{h}_{rep}", bufs=12) as hp, \
                         tc.tile_pool(name=f"hopm{h}_{rep}", bufs=8) as hm, \
                         tc.tile_pool(name=f"hopb{h}_{rep}", bufs=3) as hb, \
                         tc.tile_pool(name=f"hopp{h}_{rep}", bufs=2,
                                      space="PSUM") as pp:
                        for (g0, g1) in bgroups:
                            nxw = hb.tile([P, GB * D], BF16, tag="nxw")
                            for b in range(g0, g1):
                                aggT_ps = pp.tile([P, P], F32, tag="agg",
                                                  space="PSUM")
                                nch = int(C_b[b])
                                for k in range(nch):
                                    i = int(cob[b]) + k
                                    if no_gather:
                                        msg = cmsg[:, :D]
                                    else:
                                        msg = hp.tile([P, D], BF16, tag="msg")
                                        _queue(nc.gpsimd.indirect_dma_start(
                                            out=msg[:], out_offset=None,
                                            in_=table[:],
                                            in_offset=bass.IndirectOffsetOnAxis(
                                                ap=esrc_sb[:, i:i + 1], axis=0)))
                                    mask = hm.tile([P, P], BF16, tag="mask")
                                    nc.vector.tensor_scalar(
                                        out=mask[:], in0=iF[:],
                                        scalar1=edst_sb[:, i:i + 1],
                                        scalar2=ew_sb[:, i:i + 1],
                                        op0=mybir.AluOpType.is_equal,
                                        op1=mybir.AluOpType.mult)
                                    nc.tensor.matmul(
                                        aggT_ps[:],
                                        lhsT=msg[:],
                                        rhs=mask[:],
                                        start=(k == 0), stop=(k == nch - 1))
                                aggT = hb.tile([P, P], BF16, tag="aggT")
                                nc.vector.tensor_copy(aggT[:], aggT_ps[:])
                                hx_ps = pp.tile([P, P], F32, tag="hx",
                                                space="PSUM")
                                nc.tensor.matmul(hx_ps[:], lhsT=Ws[h][:],
                                                 rhs=xT_cur[:, b * P:(b + 1) * P],
                                                 start=True, stop=False)
                                nc.tensor.matmul(hx_ps[:], lhsT=Wn[h][:],
                                                 rhs=aggT[:],
                                                 start=False, stop=True)
                                nc.scalar.activation(
                                    xT_nxt[:, b * P:(b + 1) * P], hx_ps[:],
                                    mybir.ActivationFunctionType.Relu,
                                    bias=bc[:, h:h + 1])
                                nx_ps = pp.tile([P, P], BF16, tag="nx",
                                                space="PSUM")
                                nc.tensor.transpose(
                                    out=nx_ps[:],
                                    in_=xT_nxt[:, b * P:(b + 1) * P],
                                    identity=idn[:])
                                nc.vector.tensor_copy(
                                    nxw[:, (b - g0) * D:(b - g0 + 1) * D],
                                    nx_ps[:])
                            nc.sync.dma_start(
                                out=newx_loc[h][g0 * P:g1 * P, :].rearrange(
                                    "(g p) d -> p g d", p=P),
                                in_=nxw[:, :(g1 - g0) * D].rearrange(
                                    "p (g d) -> p g d", d=D))
                    if no_ag:
                        nc.sync.dma_start(out=x_shared[h][0:NPC, :],
                                          in_=newx_loc[h][:, :])
                    else:
                        nc.gpsimd.collective_compute(
                            "AllGather", mybir.AluOpType.bypass,
                            replica_groups=rg,
                            ins=[newx_loc[h][:]],
                            outs=[x_shared[h][:]])

                with tc.tile_pool(name=f"fin_{rep}", bufs=2) as fp:
                    for (j0, j1) in qgroups:
                        jq = j1 - j0
                        if no_gather:
                            x2g = cmsg
                        else:
                            x2g = fp.tile([P, MQ * D], BF16, tag="x2g")
                            for j in range(j0, j1):
                                _queue(nc.gpsimd.indirect_dma_start(
                                    out=x2g[:, (j - j0) * D:(j - j0 + 1) * D],
                                    out_offset=None,
                                    in_=x_shared[HOPS - 1][:],
                                    in_offset=bass.IndirectOffsetOnAxis(
                                        ap=qnode_sb[:, j:j + 1], axis=0)))
                        efw = fp.tile([P, MQ * D], BF16, tag="efw")
                        nc.sync.dma_start(
                            out=efw[:, :jq * D].rearrange(
                                "p (g d) -> p g d", d=D),
                            in_=effq[j0 * P:j1 * P, :].rearrange(
                                "(g p) d -> p g d", p=P))
                        osb = fp.tile([P, MQ * D], F32, tag="osb")
                        nc.vector.tensor_add(osb[:, :jq * D], x2g[:, :jq * D],
                                             efw[:, :jq * D])
                        nc.sync.dma_start(
                            out=out_dram[j0 * P:j1 * P, :].rearrange(
                                "(g p) d -> p g d", p=P),
                            in_=osb[:, :jq * D].rearrange(
                                "p (g d) -> p g d", d=D))
                        if timing and rep == krep - 1 and j1 == QCH:
                            pt = fp.tile([P, 1], F32, tag="probe")
                            nc.vector.tensor_copy(pt[:], osb[:, :1])
                            nc.sync.dma_start(out=probe[:, :], in_=pt[:])
    return nc


# ------------------------------------------------------------------- runner

def _build_runner(nc, n_cores, sim=False):
    install_neuronx_cc_hook()
    partition_name = nc.partition_id_tensor.name if nc.partition_id_tensor else None

    in_names, out_names, out_avals = [], [], []
    for alloc in nc.m.functions[0].allocations:
        if not isinstance(alloc, mybir.MemoryLocationSet):
            continue
        name = alloc.memorylocations[0].name
        if alloc.kind == "ExternalInput":
            if name != partition_name:
                in_names.append(name)
        elif alloc.kind == "ExternalOutput":
            out_names.append(name)
            out_avals.append(jax.core.ShapedArray(
                tuple(alloc.tensor_shape), mybir.dt.np(alloc.dtype)))

    n_params = len(in_names)
    n_outs = len(out_avals)
    all_in_names = list(in_names) + list(out_names)
    if partition_name is not None:
        all_in_names.append(partition_name)

    def _body(*args):
        operands = list(args)
        if partition_name is not None:
            operands.append(partition_id_tensor())
        outs = _bass_exec_p.bind(
            *operands,
            out_avals=tuple(out_avals),
            in_names=tuple(all_in_names),
            out_names=tuple(out_names),
            lowering_input_output_aliases=(),
            sim_require_finite=True,
            sim_require_nnan=True,
            nc=nc,
        )
        return tuple(outs)

    devices = (jax.devices("cpu") if sim else jax.devices())[:n_cores]
    mesh = Mesh(np.asarray(devices), ("core",))
    in_specs = (PartitionSpec("core"),) * (n_params + n_outs)
    out_specs = (PartitionSpec("core"),) * n_outs
    sharded = jax.jit(
        shard_map(_body, mesh=mesh, in_specs=in_specs, out_specs=out_specs,
                  check_rep=False),
        keep_unused=True,
    )

    def _concat(in_maps):
        per_core = [[np.asarray(m[name]) for name in in_names] for m in in_maps]
        concat_in = [
            np.concatenate([per_core[c][i] for c in range(n_cores)], axis=0)
            for i in range(n_params)
        ]
        concat_zeros = [
            np.zeros((n_cores * av.shape[0], *av.shape[1:]), av.dtype)
            for av in out_avals
        ]
        return concat_in, concat_zeros

    def run(in_maps):
        concat_in, concat_zeros = _concat(in_maps)
        out_arrs = sharded(*concat_in, *concat_zeros)
        jax.block_until_ready(out_arrs)
        return [
            {name: np.asarray(out_arrs[i]).reshape(
                n_cores, *out_avals[i].shape)[c]
             for i, name in enumerate(out_names)}
            for c in range(n_cores)
        ]

    def timeit(in_maps, reps=1):
        concat_in, concat_zeros = _concat(in_maps)
        times = []
        for _ in range(reps):
            t0 = time.perf_counter()
            out_arrs = sharded(*concat_in, *concat_zeros)
            jax.block_until_ready(out_arrs)
            times.append(time.perf_counter() - t0)
        return out_arrs, times

    return run, timeit


# ------------------------------------------------------------------- kernel

def kernel(**inputs):
    gx = np.asarray(inputs["graph_x"])
    cfg = dict(
        N=gx.shape[0],
        E=np.asarray(inputs["edge_index"]).shape[1],
        D=gx.shape[1],
        NEFF=np.asarray(inputs["effect_emb"]).shape[0],
        Q=np.asarray(inputs["x_nodes"]).shape[0],
        C=8,
        B=-(-gx.shape[0] // (8 * P)),
        HOPS=np.asarray(inputs["W_self"]).shape[0],
    )
    meta, in_maps = _prep(inputs, cfg)
    nc = _build_nc(meta)
    _split_wide_waits(nc, 1)
    run, _ = _build_runner(nc, cfg["C"], sim=os.environ.get("KSIM") == "1")
    results = run(in_maps)

    C, QPC, D, Q = cfg["C"], meta["QPC"], cfg["D"], cfg["Q"]
    out = np.empty((Q, D), np.float32)
    for c in range(C):
        lo, hi = c * QPC, min((c + 1) * QPC, Q)
        out[lo:hi] = results[c]["out"][:hi - lo]
    return out
